# revision 45
# baseline (speedup 1.0000x reference)
"""Multi-head attention (B=2, S=2048, D=1024, H=16, DK=64) on 8 Trainium2 cores.

Sharding: 8 cores x (1 batch, 4 heads) each.  Core c handles batch c//4,
heads [4*(c%4) : 4*(c%4)+4].  Each core computes its heads' slice of the
output projection (rows of Wo for its heads); the host sums the 4 partial
outputs per batch and adds the bias.

Per-core dataflow (all matmul inputs bf16, PSUM accumulation fp32):
  - host supplies q/k/v pre-transposed per batch: qT/kT/vT [D=1024, S=2048],
    loaded in per-128-row chunks so projections start while DMA streams
  - qhT/khT [dk2=128, S] per head-pair via PE (weights stationary)
  - vh natural [S, dk4] via PE (vT chunks stationary), with a ones column
    appended per head for softmax row sums
  - scoresT[m, q] per head = khT.T-chunk @ qhT  (K=64)
  - attnT = exp(scoresT / 8) on ACT straight out of PSUM (no max-subtract:
    inputs are unit-normal with 0.02-scaled weights so |scores/8| < ~6)
  - outT(+sums) = vh_aug.T-chunk @ attnT accumulated over m (M=65)
  - normalize: out / sums via a K=1 broadcast matmul of 1/sums and a DVE
    multiply
  - partial = outT2.T-chunk @ Wo-rows accumulated over head pairs

The PE (tensor engine) is the global bottleneck (~164us of matmul
streaming + ~40us of LDWEIGHTS exposure per core), so the schedule is
organized around keeping it fed continuously:
  - input DMAs are chunked and issue-ordered so each projection group's
    data lands just before the PE needs it (attention unit 0 starts while
    kT/vT halves are still streaming, consuming the remaining projection
    work as its fill queue)
  - remaining projections and the output projection drain as ~0.5us
    granules, one per m-iteration, between attn@v and the next scores
  - the last 1024-col unit is split 512/256/256 so its dances unlock
    outproj fills for qi 8-13 while attention still runs, keeping the
    PE p-state warm into the tail; partials DMA out as bf16
"""

import numpy as np
import ml_dtypes
from contextlib import ExitStack

import concourse.bass as bass
import concourse.tile as tile
from concourse import bacc, mybir
from concourse import bass_utils

B, S, D, H, DK = 2, 2048, 1024, 16, 64
N_CORES = 8
HPC = 4            # heads per core
PAIRS = HPC // 2   # head pairs per core
KC = D // 128      # contraction chunks over D
MC = S // 128      # m (key) chunks
QC1 = S // 1024    # 1024-wide q chunks
SC1 = S // 1024    # 1024-wide s chunks for projections
F32 = mybir.dt.float32
BF16 = mybir.dt.bfloat16
BF16_NP = ml_dtypes.bfloat16

_COMPILED = {}


def _emit(tc, qT, kT, vT, wq, wk, wv, wo, out_dram):
    nc = tc.nc
    AFT = mybir.ActivationFunctionType
    qTa, kTa, vTa = qT.ap(), kT.ap(), vT.ap()
    wqa, wka, wva, woa = wq.ap(), wk.ap(), wv.ap(), wo.ap()
    outa = out_dram.ap()

    with ExitStack() as ctx:
        big = ctx.enter_context(tc.tile_pool(name="big", bufs=1))
        att = ctx.enter_context(tc.tile_pool(name="att", bufs=12))
        dance = ctx.enter_context(tc.tile_pool(name="dance", bufs=2))
        ostage = ctx.enter_context(tc.tile_pool(name="ostage", bufs=4))
        # scores ping-pong through the 2 "pp" slots; pout accumulators,
        # projection fill groups and outproj accumulators share the 2 "po"
        # slots (fills run in early units, outproj in late units)
        ppool = ctx.enter_context(tc.tile_pool(name="psum", bufs=2, space="PSUM"))
        popool = ctx.enter_context(tc.tile_pool(name="psum_o", bufs=2, space="PSUM"))

        # ---- chunked, demand-ordered input DMAs --------------------------
        # The PE is the global bottleneck, so the DMA issue order is chosen
        # to keep it fed from ~7us on: each chunk arrives just before the
        # projection group that consumes it.  Chunks are S-column slices
        # (a projection group contracts over all of D for one S range).
        wq_sb = big.tile([128, KC, HPC * DK], BF16, tag="wq")
        wk_sb = big.tile([128, KC, HPC * DK], BF16, tag="wk")
        wv_sb = big.tile([128, KC, HPC * DK], BF16, tag="wv")
        wo_sb = big.tile([128, PAIRS, D], BF16, tag="wo")
        kT_sb = big.tile([128, KC, S], BF16, tag="kT")
        qT_sb = big.tile([128, KC, S], BF16, tag="qT")
        vT_sb = big.tile([128, KC, S], BF16, tag="vT")

        def dma_cols(dst_sb, src_ap, lo, hi):
            nc.sync.dma_start(
                dst_sb[:, :, lo:hi],
                src_ap[:, lo:hi].rearrange("(c p) s -> p c s", p=128),
            )

        def dma_rows(dst_sb, src_ap, lo, hi, klo, khi):
            nc.sync.dma_start(
                dst_sb[:, klo:khi, lo:hi],
                src_ap[klo * 128:khi * 128, lo:hi].rearrange(
                    "(c p) s -> p c s", p=128
                ),
            )

        nc.sync.dma_start(
            wk_sb[:, 0:2, :], wka[0:256].rearrange("(c p) n -> p c n", p=128)
        )
        dma_rows(kT_sb, kTa, 0, 1024, 0, 2)                            # khT sc0 kc0-1
        nc.sync.dma_start(
            wk_sb[:, 2:KC, :], wka[256:D].rearrange("(c p) n -> p c n", p=128)
        )
        dma_rows(kT_sb, kTa, 0, 1024, 2, 4)                            # khT sc0 kc2-3
        dma_rows(kT_sb, kTa, 0, 1024, 4, 8)                            # khT sc0 kc4-7
        nc.sync.dma_start(wv_sb[:], wva.rearrange("(c p) n -> p c n", p=128))
        dma_cols(vT_sb, vTa, 0, 512)                                   # vp 0-3
        nc.sync.dma_start(wq_sb[:], wqa.rearrange("(c p) n -> p c n", p=128))
        dma_rows(qT_sb, qTa, 0, 1024, 0, 4)                            # qhT sc0 kc0-3
        dma_rows(qT_sb, qTa, 0, 1024, 4, 8)                            # qhT sc0 kc4-7
        dma_cols(vT_sb, vTa, 512, 1024)                                # vp 4-7
        dma_rows(kT_sb, kTa, 1024, 2048, 0, 4)                         # khT sc1
        dma_rows(kT_sb, kTa, 1024, 2048, 4, 8)
        dma_cols(vT_sb, vTa, 1024, 1536)                               # vp 8-11
        dma_cols(vT_sb, vTa, 1536, 2048)                               # vp 12-15
        dma_rows(qT_sb, qTa, 1024, 2048, 0, 4)                         # qhT sc1 fills
        dma_rows(qT_sb, qTa, 1024, 2048, 4, 8)
        nc.sync.dma_start(wo_sb[:], woa.rearrange("(c p) d -> p c d", p=128))

        # warm the ACT exp table during the DMA phase
        warm_sb = big.tile([1, 64], BF16, tag="warm")
        nc.vector.memset(warm_sb[:], 1.0)
        nc.scalar.activation(warm_sb[:], warm_sb[:], AFT.Exp)

        # vh with a ones column per (m-chunk, head): [128, MC, HPC, 65]
        vh_sb = big.tile([128, MC, HPC, DK + 1], BF16, tag="vh")
        nc.vector.memset(vh_sb[:], 1.0)

        qhT_sb = [
            big.tile([128, S], BF16, tag=f"qhT{p}", name=f"qhT{p}")
            for p in range(PAIRS)
        ]
        khT_sb = [
            big.tile([128, S], BF16, tag=f"khT{p}", name=f"khT{p}")
            for p in range(PAIRS)
        ]
        outT2_sb = [
            big.tile([128, S], BF16, tag=f"o2{p}", name=f"o2{p}")
            for p in range(PAIRS)
        ]

        def emit_proj_qk(p, w_sb, src, dst, sc):
            """One accumulation group: dst[:, sc*1024:+1024] for pair p."""
            ps = ppool.tile([128, 1024], F32, tag="pp", name="ps_proj")
            for kc in range(KC):
                for j in range(2):
                    nc.tensor.matmul(
                        ps[:, j * 512:(j + 1) * 512],
                        w_sb[:, kc, p * 128:(p + 1) * 128],
                        src[:, kc, sc * 1024 + j * 512: sc * 1024 + (j + 1) * 512],
                        start=(kc == 0),
                        stop=(kc == KC - 1),
                    )
            nc.vector.tensor_copy(dst[:, sc * 1024:(sc + 1) * 1024], ps[:])

        def emit_proj_v(mc):
            ps = ppool.tile([128, HPC * DK], F32, tag="pp", name="ps_v")
            for kc in range(KC):
                nc.tensor.matmul(
                    ps[:],
                    vT_sb[:, kc, mc * 128:(mc + 1) * 128],
                    wv_sb[:, kc, :],
                    start=(kc == 0),
                    stop=(kc == KC - 1),
                )
            nc.vector.tensor_copy(
                vh_sb[:, mc, :, 0:DK],
                ps[:].rearrange("p (h k) -> p h k", k=DK),
            )

        def emit_outproj_half(qi, j, use_act=False):
            po = popool.tile([128, 512], F32, tag="po", name="po")
            for p in range(PAIRS):
                nc.tensor.matmul(
                    po[:],
                    outT2_sb[p][:, qi * 128:(qi + 1) * 128],
                    wo_sb[:, p, j * 512:(j + 1) * 512],
                    start=(p == 0),
                    stop=(p == PAIRS - 1),
                )
            so = ostage.tile([128, 512], BF16, tag="so", name="so")
            if use_act:
                # in the tail ACT is idle; splitting the psum->sbuf casts
                # across ACT and DVE halves the cast-bound drain
                nc.scalar.activation(so[:], po[:], AFT.Copy)
            else:
                nc.vector.tensor_copy(so[:], po[:])
            nc.sync.dma_start(
                outa[qi * 128:(qi + 1) * 128, j * 512:(j + 1) * 512], so[:]
            )

        # ---- upfront PE work: only what unit 0's first half (mc 0-7)
        # needs; everything else drains as unit-0 fills so the PE overlaps
        # attention with the still-streaming input DMAs -------------------
        emit_proj_qk(0, wk_sb, kT_sb, khT_sb[0], 0)
        # pair-1 khT sc0 needs only kTa (already on chip) — it fills the
        # DMA window between kTa and vTa/qTa arrivals
        emit_proj_qk(1, wk_sb, kT_sb, khT_sb[1], 0)
        for mc in range(0, 4):
            emit_proj_v(mc)
        emit_proj_qk(0, wq_sb, qT_sb, qhT_sb[0], 0)

        def vp_granules(mc):
            state = {}
            def granule(i, state=state, mc=mc):
                if i == 0:
                    state["ps"] = ppool.tile(
                        [128, HPC * DK], F32, tag="pp", name="ps_v"
                    )
                ps = state["ps"]
                for kc in range(4 * i, 4 * i + 4):
                    nc.tensor.matmul(
                        ps[:],
                        vT_sb[:, kc, mc * 128:(mc + 1) * 128],
                        wv_sb[:, kc, :],
                        start=(kc == 0),
                        stop=(kc == KC - 1),
                    )
                if i == 1:
                    nc.vector.tensor_copy(
                        vh_sb[:, mc, :, 0:DK],
                        ps[:].rearrange("p (h k) -> p h k", k=DK),
                    )
            return [lambda i=i: granule(i) for i in range(2)]



        # Remaining projection groups drain as fill work inside the
        # attention units, one 512-col matmul granule per mc step, so a
        # ~0.5us slice of independent PE work sits between every attn@v
        # and the next scores matmul (absorbing the exp psum ping-pong
        # wait that otherwise stalls the PE).
        def proj_granules(p, w_sb, src, dst, sc):
            """One granule = both 512-col j halves of one kc chunk: the two
            matmuls share a stationary, so the second's LDWEIGHTS dedupes to
            ~19ns instead of paying the ~100ns weight-FIFO exposure."""
            state = {}
            def granule(kc, state=state, p=p, w_sb=w_sb, src=src, dst=dst, sc=sc):
                if kc == 0:
                    state["ps"] = popool.tile([128, 1024], F32, tag="po", name="ps_fq")
                ps = state["ps"]
                for j in range(2):
                    nc.tensor.matmul(
                        ps[:, j * 512:(j + 1) * 512],
                        w_sb[:, kc, p * 128:(p + 1) * 128],
                        src[:, kc, sc * 1024 + j * 512: sc * 1024 + (j + 1) * 512],
                        start=(kc == 0),
                        stop=(kc == KC - 1),
                    )
                if kc == KC - 1:
                    nc.vector.tensor_copy(dst[:, sc * 1024:(sc + 1) * 1024], ps[:])
            return [lambda kc=kc: granule(kc) for kc in range(KC)]

        # unit-0 fill stream, in DMA arrival order: vp 4-7, khT sc1, vp 8-15
        pro_fills = []
        for mc in range(4, 8):
            pro_fills += vp_granules(mc)
        pro_fills += proj_granules(0, wk_sb, kT_sb, khT_sb[0], 1)
        for mc in range(8, MC):
            pro_fills += vp_granules(mc)

        # fq_early: projections units 2-4 need soon; fq_mid: qhT pair-1 sc1
        # is not read until unit 6, so it drains in the otherwise
        # fill-starved units 4-5 instead of adding to the PE-bound early
        # units; fq_late: outproj halves appended as dances unlock them
        fq_early = (
            proj_granules(0, wq_sb, qT_sb, qhT_sb[0], 1)
            + proj_granules(1, wk_sb, kT_sb, khT_sb[1], 1)
            + proj_granules(1, wq_sb, qT_sb, qhT_sb[1], 0)
        )
        fq_mid = proj_granules(1, wq_sb, qT_sb, qhT_sb[1], 1)
        fq_late = []

        # ---- attention: flat software-pipelined stream -------------------
        # Per unit (head, q-range): 16 scores+exp iterations; attn@v lags
        # the exp stream by 2 so an exp wait never sits between a scores
        # matmul and the next unit's scores.  The last two attn@v matmuls
        # and the normalization of unit u are carried into unit u+1's first
        # slots (pout is double-buffered, so no serialization).
        def emit_av(st, mc):
            p, hh, h = st["p"], st["hh"], 2 * st["p"] + st["hh"]
            qw = st["qw"]
            for j in range((qw + 511) // 512):
                w = min(512, qw - j * 512)
                nc.tensor.matmul(
                    st["pout"][:, j * 512:j * 512 + w],
                    vh_sb[:, mc, h, :],
                    st["at"][mc][:, j * 512:j * 512 + w],
                    start=(mc == 0),
                    stop=(mc == MC - 1),
                )
            del st["at"][mc]

        def emit_dance(st):
            p, hh = st["p"], st["hh"]
            hlo, hhi = hh * 64, hh * 64 + 64
            qlo, qw = st["qlo"], st["qw"]
            pout = st["pout"]
            sums = dance.tile([1, qw], F32, tag="sums", name="sums")
            nc.vector.tensor_copy(sums[:], pout[64:65, :])
            rcp32 = dance.tile([1, qw], F32, tag="rcp32", name="rcp32")
            nc.vector.reciprocal_approx_fast(rcp32[:], sums[:])
            rcpb = dance.tile([64, qw], F32, tag="rcpb", name="rcpb")
            nc.gpsimd.partition_broadcast(rcpb[:], rcp32[:])
            nc.vector.tensor_tensor(
                outT2_sb[p][hlo:hhi, qlo:qlo + qw],
                pout[0:64, :],
                rcpb[:],
                mybir.AluOpType.mult,
            )

        def attention_unit(p, hh, qlo, qw, fills, carry, unit_idx,
                           pop_schedule=None):
            st = {
                "p": p, "hh": hh, "qlo": qlo, "qw": qw, "at": {},
                "pout": popool.tile([65, qw], F32, tag="po", name="pout"),
            }
            for mc in range(MC):
                # scores first so exp(mc) can start as early as possible;
                # the previous unit's carried attn@v tail follows (it only
                # gates pout/attnT slots needed a couple of steps later)
                ps = ppool.tile([128, qw], F32, tag="pp", name="ps_sc")
                for j in range((qw + 511) // 512):
                    w = min(512, qw - j * 512)
                    nc.tensor.matmul(
                        ps[:, j * 512:j * 512 + w],
                        khT_sb[p][hh * 64:hh * 64 + 64, mc * 128:(mc + 1) * 128],
                        qhT_sb[p][hh * 64:hh * 64 + 64,
                                  qlo + j * 512: qlo + j * 512 + w],
                        start=True,
                        stop=True,
                    )
                at = att.tile([128, qw], BF16, tag="attnT", name="at")
                nc.scalar.activation(at[:], ps[:], AFT.Exp, scale=0.125)
                st["at"][mc] = at
                for _ in range(2):
                    if carry:
                        carry.pop(0)()
                if mc >= 2:
                    emit_av(st, mc - 2)
                # fills only from mc 2 on: a fill popped at mc 0/1 could read
                # outT2 whose producing dance is carried into this unit's
                # mc 1 slot and so is not yet emitted (stale read)
                if pop_schedule is not None:
                    n_pops = pop_schedule(mc)
                elif mc >= 2:
                    # double pop at mc 15: the extra fill absorbs the
                    # unit-boundary wait on the previous unit's exp tail
                    n_pops = 2 if mc == 15 else 1
                else:
                    n_pops = 0
                for _ in range(n_pops):
                    if fills:
                        fills.pop(0)()
            return [
                lambda: emit_av(st, MC - 2),
                lambda: emit_av(st, MC - 1),
                lambda: emit_dance(st),
            ]

        # pair-outer unit order: pair-1 attention starts at unit 4, so its
        # projections drain as fills through units 0-3.  The last 1024-col
        # unit is split into two 512-col halves so the first half's dance
        # unlocks outproj qi 8-11 while the second half still runs.
        units = [(0, 0, 0, 1024), (0, 1, 0, 1024),
                 (0, 0, 1024, 1024), (0, 1, 1024, 1024),
                 (1, 0, 0, 1024), (1, 1, 0, 1024),
                 (1, 0, 1024, 1024),
                 (1, 1, 1024, 512), (1, 1, 1536, 256), (1, 1, 1792, 256)]
        def u0_pops(mc):
            if mc < 2:
                return 0
            return 4 if mc < 4 else 2

        def mid_pops(mc):
            return 1 if mc in (3, 7, 11, 14) else 0

        carry = []
        for u, (p, hh, qlo, qw) in enumerate(units):
            if u == 0:
                fills, sched = pro_fills, u0_pops
            elif u <= 3:
                fills, sched = fq_early, None
            elif u <= 5:
                fills, sched = fq_mid, mid_pops
            else:
                fills, sched = fq_late, None
            carry = attention_unit(p, hh, qlo, qw, fills, carry, u,
                                   pop_schedule=sched)
            if u == 5:
                for qi in range(0, 8):
                    for j in range(2):
                        fq_late.append(
                            lambda qi=qi, j=j: emit_outproj_half(qi, j)
                        )
            if u == 7:
                for qi in range(8, 12):
                    for j in range(2):
                        fq_late.append(
                            lambda qi=qi, j=j: emit_outproj_half(qi, j)
                        )
            if u == 8:
                for qi in range(12, 14):
                    for j in range(2):
                        fq_late.append(
                            lambda qi=qi, j=j: emit_outproj_half(qi, j)
                        )
        # tail: qi 14-15's pair-0 outproj half only needs outT2[0] (ready
        # since unit 3), so it runs inside the last unit's exp-tail wait in
        # the freed scores psum ring; pair-1 closes the accumulation after
        # the final dance
        po_tail = []
        for qi in (14, 15):
            po = ppool.tile([128, 1024], F32, tag="pp", name="po_t")
            for j in range(2):
                nc.tensor.matmul(
                    po[:, j * 512:(j + 1) * 512],
                    outT2_sb[0][:, qi * 128:(qi + 1) * 128],
                    wo_sb[:, 0, j * 512:(j + 1) * 512],
                    start=True,
                    stop=False,
                )
            po_tail.append((qi, po))
        for f in carry:
            f()
        for q in (fq_early, fq_mid, fq_late):
            while q:
                q.pop(0)()
        for qi, po in po_tail:
            for j in range(2):
                nc.tensor.matmul(
                    po[:, j * 512:(j + 1) * 512],
                    outT2_sb[1][:, qi * 128:(qi + 1) * 128],
                    wo_sb[:, 1, j * 512:(j + 1) * 512],
                    start=False,
                    stop=True,
                )
            for j, use_act in ((0, False), (1, True)):
                so = ostage.tile([128, 512], BF16, tag="so", name="so")
                if use_act:
                    nc.scalar.activation(so[:], po[:, j * 512:(j + 1) * 512],
                                         AFT.Copy)
                else:
                    nc.vector.tensor_copy(so[:], po[:, j * 512:(j + 1) * 512])
                nc.sync.dma_start(
                    outa[qi * 128:(qi + 1) * 128, j * 512:(j + 1) * 512], so[:]
                )


def build_program():
    nc = bacc.Bacc(
        "TRN2",
        target_bir_lowering=False,
        debug=False,
        enable_asserts=False,
        num_devices=N_CORES,
    )
    qT = nc.dram_tensor("qT", [D, S], BF16, kind="ExternalInput")
    kT = nc.dram_tensor("kT", [D, S], BF16, kind="ExternalInput")
    vT = nc.dram_tensor("vT", [D, S], BF16, kind="ExternalInput")
    wq = nc.dram_tensor("wq", [D, HPC * DK], BF16, kind="ExternalInput")
    wk = nc.dram_tensor("wk", [D, HPC * DK], BF16, kind="ExternalInput")
    wv = nc.dram_tensor("wv", [D, HPC * DK], BF16, kind="ExternalInput")
    wo = nc.dram_tensor("wo", [HPC * DK, D], BF16, kind="ExternalInput")
    out = nc.dram_tensor("out", [S, D], BF16, kind="ExternalOutput")
    with tile.TileContext(nc) as tc:
        _emit(tc, qT, kT, vT, wq, wk, wv, wo, out)
    nc.compile()
    return nc


def _get_program():
    if "nc" not in _COMPILED:
        _COMPILED["nc"] = build_program()
    return _COMPILED["nc"]


def make_in_maps(q, k, v, Wq, Wk, Wv, Wo):
    """Shard FULL fp32 inputs into per-core bf16 input maps."""
    q, k, v = (np.asarray(x, np.float32) for x in (q, k, v))
    Wq, Wk, Wv, Wo = (np.asarray(x, np.float32) for x in (Wq, Wk, Wv, Wo))
    qT = [np.ascontiguousarray(q[b].T).astype(BF16_NP) for b in range(B)]
    kT = [np.ascontiguousarray(k[b].T).astype(BF16_NP) for b in range(B)]
    vT = [np.ascontiguousarray(v[b].T).astype(BF16_NP) for b in range(B)]
    in_maps = []
    for c in range(N_CORES):
        b, g = divmod(c, N_CORES // B)
        heads = range(HPC * g, HPC * g + HPC)
        wq_c = np.concatenate([Wq[h] for h in heads], axis=1).astype(BF16_NP)
        wk_c = np.concatenate([Wk[h] for h in heads], axis=1).astype(BF16_NP)
        wv_c = np.concatenate([Wv[h] for h in heads], axis=1).astype(BF16_NP)
        wo_c = np.concatenate(
            [Wo[h * DK:(h + 1) * DK] for h in heads], axis=0
        ).astype(BF16_NP)
        in_maps.append({
            "qT": qT[b], "kT": kT[b], "vT": vT[b],
            "wq": np.ascontiguousarray(wq_c),
            "wk": np.ascontiguousarray(wk_c),
            "wv": np.ascontiguousarray(wv_c),
            "wo": np.ascontiguousarray(wo_c),
        })
    return in_maps


def run_on_hw(in_maps, trace=False):
    nc = _get_program()
    return bass_utils.run_bass_kernel_spmd(
        nc, in_maps, list(range(N_CORES)), trace=trace
    )


def kernel(q, k, v, Wq, Wk, Wv, Wo, bo):
    in_maps = make_in_maps(q, k, v, Wq, Wk, Wv, Wo)
    res = run_on_hw(in_maps)
    bo = np.asarray(bo, np.float32)
    parts = [np.asarray(r["out"]).astype(np.float32) for r in res.results]
    out = np.empty((B, S, D), np.float32)
    per_b = N_CORES // B
    for b in range(B):
        out[b] = np.sum(parts[b * per_b:(b + 1) * per_b], axis=0) + bo
    return out



# revision 46
# speedup vs baseline: 1.0101x; 1.0101x over previous
"""Multi-head attention (B=2, S=2048, D=1024, H=16, DK=64) on 8 Trainium2 cores.

Sharding: 8 cores x (1 batch, 4 heads) each.  Core c handles batch c//4,
heads [4*(c%4) : 4*(c%4)+4].  Each core computes its heads' slice of the
output projection (rows of Wo for its heads); the host sums the 4 partial
outputs per batch and adds the bias.

Per-core dataflow (all matmul inputs bf16, PSUM accumulation fp32):
  - host supplies q/k/v pre-transposed per batch: qT/kT/vT [D=1024, S=2048],
    loaded in per-128-row chunks so projections start while DMA streams
  - qhT/khT [dk2=128, S] per head-pair via PE (weights stationary)
  - vh natural [S, dk4] via PE (vT chunks stationary), with a ones column
    appended per head for softmax row sums
  - scoresT[m, q] per head = khT.T-chunk @ qhT  (K=64)
  - attnT = exp(scoresT / 8) on ACT straight out of PSUM (no max-subtract:
    inputs are unit-normal with 0.02-scaled weights so |scores/8| < ~6)
  - outT(+sums) = vh_aug.T-chunk @ attnT accumulated over m (M=65)
  - normalize: out / sums via a K=1 broadcast matmul of 1/sums and a DVE
    multiply
  - partial = outT2.T-chunk @ Wo-rows accumulated over head pairs

The PE (tensor engine) is the global bottleneck (~164us of matmul
streaming + ~40us of LDWEIGHTS exposure per core), so the schedule is
organized around keeping it fed continuously:
  - input DMAs are chunked and issue-ordered so each projection group's
    data lands just before the PE needs it (attention unit 0 starts while
    kT/vT halves are still streaming, consuming the remaining projection
    work as its fill queue)
  - remaining projections and the output projection drain as ~0.5us
    granules, one per m-iteration, between attn@v and the next scores
  - the last 1024-col unit is split 512/256/256 so its dances unlock
    outproj fills for qi 8-13 while attention still runs, keeping the
    PE p-state warm into the tail; partials DMA out as bf16
"""

import numpy as np
import ml_dtypes
from contextlib import ExitStack

import concourse.bass as bass
import concourse.tile as tile
from concourse import bacc, mybir
from concourse import bass_utils

B, S, D, H, DK = 2, 2048, 1024, 16, 64
N_CORES = 8
HPC = 4            # heads per core
PAIRS = HPC // 2   # head pairs per core
KC = D // 128      # contraction chunks over D
MC = S // 128      # m (key) chunks
QC1 = S // 1024    # 1024-wide q chunks
SC1 = S // 1024    # 1024-wide s chunks for projections
F32 = mybir.dt.float32
BF16 = mybir.dt.bfloat16
BF16_NP = ml_dtypes.bfloat16

_COMPILED = {}


def _emit(tc, qT, kT, vT, wq, wk, wv, wo, out_dram):
    nc = tc.nc
    AFT = mybir.ActivationFunctionType
    qTa, kTa, vTa = qT.ap(), kT.ap(), vT.ap()
    wqa, wka, wva, woa = wq.ap(), wk.ap(), wv.ap(), wo.ap()
    outa = out_dram.ap()

    with ExitStack() as ctx:
        big = ctx.enter_context(tc.tile_pool(name="big", bufs=1))
        att = ctx.enter_context(tc.tile_pool(name="att", bufs=12))
        dance = ctx.enter_context(tc.tile_pool(name="dance", bufs=2))
        ostage = ctx.enter_context(tc.tile_pool(name="ostage", bufs=4))
        # scores ping-pong through the 2 "pp" slots; pout accumulators,
        # projection fill groups and outproj accumulators share the 2 "po"
        # slots (fills run in early units, outproj in late units)
        ppool = ctx.enter_context(tc.tile_pool(name="psum", bufs=2, space="PSUM"))
        popool = ctx.enter_context(tc.tile_pool(name="psum_o", bufs=2, space="PSUM"))

        # ---- chunked, demand-ordered input DMAs --------------------------
        # The PE is the global bottleneck, so the DMA issue order is chosen
        # to keep it fed from ~7us on: each chunk arrives just before the
        # projection group that consumes it.  Chunks are S-column slices
        # (a projection group contracts over all of D for one S range).
        wq_sb = big.tile([128, KC, HPC * DK], BF16, tag="wq")
        wk_sb = big.tile([128, KC, HPC * DK], BF16, tag="wk")
        wv_sb = big.tile([128, KC, HPC * DK], BF16, tag="wv")
        wo_sb = big.tile([128, PAIRS, D], BF16, tag="wo")
        kT_sb = big.tile([128, KC, S], BF16, tag="kT")
        qT_sb = big.tile([128, KC, S], BF16, tag="qT")
        vT_sb = big.tile([128, KC, S], BF16, tag="vT")

        def dma_cols(dst_sb, src_ap, lo, hi):
            nc.sync.dma_start(
                dst_sb[:, :, lo:hi],
                src_ap[:, lo:hi].rearrange("(c p) s -> p c s", p=128),
            )

        def dma_rows(dst_sb, src_ap, lo, hi, klo, khi):
            nc.sync.dma_start(
                dst_sb[:, klo:khi, lo:hi],
                src_ap[klo * 128:khi * 128, lo:hi].rearrange(
                    "(c p) s -> p c s", p=128
                ),
            )

        nc.sync.dma_start(
            wk_sb[:, 0:2, :], wka[0:256].rearrange("(c p) n -> p c n", p=128)
        )
        dma_rows(kT_sb, kTa, 0, 1024, 0, 2)                            # khT sc0 kc0-1
        nc.sync.dma_start(
            wk_sb[:, 2:KC, :], wka[256:D].rearrange("(c p) n -> p c n", p=128)
        )
        dma_rows(kT_sb, kTa, 0, 1024, 2, 4)                            # khT sc0 kc2-3
        dma_rows(kT_sb, kTa, 0, 1024, 4, 8)                            # khT sc0 kc4-7
        nc.sync.dma_start(wv_sb[:], wva.rearrange("(c p) n -> p c n", p=128))
        dma_cols(vT_sb, vTa, 0, 512)                                   # vp 0-3
        nc.sync.dma_start(wq_sb[:], wqa.rearrange("(c p) n -> p c n", p=128))
        dma_rows(qT_sb, qTa, 0, 1024, 0, 4)                            # qhT sc0 kc0-3
        dma_rows(qT_sb, qTa, 0, 1024, 4, 8)                            # qhT sc0 kc4-7
        dma_cols(vT_sb, vTa, 512, 1024)                                # vp 4-7
        dma_rows(kT_sb, kTa, 1024, 2048, 0, 4)                         # khT sc1
        dma_rows(kT_sb, kTa, 1024, 2048, 4, 8)
        dma_cols(vT_sb, vTa, 1024, 1536)                               # vp 8-11
        dma_cols(vT_sb, vTa, 1536, 2048)                               # vp 12-15
        dma_rows(qT_sb, qTa, 1024, 2048, 0, 4)                         # qhT sc1 fills
        dma_rows(qT_sb, qTa, 1024, 2048, 4, 8)
        nc.sync.dma_start(wo_sb[:], woa.rearrange("(c p) d -> p c d", p=128))

        # warm the ACT exp table during the DMA phase
        warm_sb = big.tile([1, 64], BF16, tag="warm")
        nc.vector.memset(warm_sb[:], 1.0)
        nc.scalar.activation(warm_sb[:], warm_sb[:], AFT.Exp)

        # vh with a ones column per (m-chunk, head): [128, MC, HPC, 65]
        vh_sb = big.tile([128, MC, HPC, DK + 1], BF16, tag="vh")
        nc.vector.memset(vh_sb[:], 1.0)

        qhT_sb = [
            big.tile([128, S], BF16, tag=f"qhT{p}", name=f"qhT{p}")
            for p in range(PAIRS)
        ]
        khT_sb = [
            big.tile([128, S], BF16, tag=f"khT{p}", name=f"khT{p}")
            for p in range(PAIRS)
        ]
        outT2_sb = [
            big.tile([128, S], BF16, tag=f"o2{p}", name=f"o2{p}")
            for p in range(PAIRS)
        ]

        def emit_proj_qk(p, w_sb, src, dst, sc):
            """One accumulation group: dst[:, sc*1024:+1024] for pair p."""
            ps = ppool.tile([128, 1024], F32, tag="pp", name="ps_proj")
            for kc in range(KC):
                for j in range(2):
                    nc.tensor.matmul(
                        ps[:, j * 512:(j + 1) * 512],
                        w_sb[:, kc, p * 128:(p + 1) * 128],
                        src[:, kc, sc * 1024 + j * 512: sc * 1024 + (j + 1) * 512],
                        start=(kc == 0),
                        stop=(kc == KC - 1),
                    )
            nc.vector.tensor_copy(dst[:, sc * 1024:(sc + 1) * 1024], ps[:])

        def emit_proj_v(mc):
            ps = ppool.tile([128, HPC * DK], F32, tag="pp", name="ps_v")
            for kc in range(KC):
                nc.tensor.matmul(
                    ps[:],
                    vT_sb[:, kc, mc * 128:(mc + 1) * 128],
                    wv_sb[:, kc, :],
                    start=(kc == 0),
                    stop=(kc == KC - 1),
                )
            nc.vector.tensor_copy(
                vh_sb[:, mc, :, 0:DK],
                ps[:].rearrange("p (h k) -> p h k", k=DK),
            )

        def emit_outproj_half(qi, j, use_act=False):
            po = popool.tile([128, 512], F32, tag="po", name="po")
            for p in range(PAIRS):
                nc.tensor.matmul(
                    po[:],
                    outT2_sb[p][:, qi * 128:(qi + 1) * 128],
                    wo_sb[:, p, j * 512:(j + 1) * 512],
                    start=(p == 0),
                    stop=(p == PAIRS - 1),
                )
            so = ostage.tile([128, 512], BF16, tag="so", name="so")
            if use_act:
                # in the tail ACT is idle; splitting the psum->sbuf casts
                # across ACT and DVE halves the cast-bound drain
                nc.scalar.activation(so[:], po[:], AFT.Copy)
            else:
                nc.vector.tensor_copy(so[:], po[:])
            nc.sync.dma_start(
                outa[qi * 128:(qi + 1) * 128, j * 512:(j + 1) * 512], so[:]
            )

        # ---- upfront PE work: only what unit 0's first half (mc 0-7)
        # needs; everything else drains as unit-0 fills so the PE overlaps
        # attention with the still-streaming input DMAs -------------------
        emit_proj_qk(0, wk_sb, kT_sb, khT_sb[0], 0)
        # pair-1 khT sc0 needs only kTa (already on chip) — it fills the
        # DMA window between kTa and vTa/qTa arrivals
        emit_proj_qk(1, wk_sb, kT_sb, khT_sb[1], 0)
        for mc in range(0, 4):
            emit_proj_v(mc)
        emit_proj_qk(0, wq_sb, qT_sb, qhT_sb[0], 0)

        def vp_granules(mc):
            state = {}
            def granule(i, state=state, mc=mc):
                if i == 0:
                    state["ps"] = ppool.tile(
                        [128, HPC * DK], F32, tag="pp", name="ps_v"
                    )
                ps = state["ps"]
                for kc in range(4 * i, 4 * i + 4):
                    nc.tensor.matmul(
                        ps[:],
                        vT_sb[:, kc, mc * 128:(mc + 1) * 128],
                        wv_sb[:, kc, :],
                        start=(kc == 0),
                        stop=(kc == KC - 1),
                    )
                if i == 1:
                    nc.vector.tensor_copy(
                        vh_sb[:, mc, :, 0:DK],
                        ps[:].rearrange("p (h k) -> p h k", k=DK),
                    )
            return [lambda i=i: granule(i) for i in range(2)]



        # Remaining projection groups drain as fill work inside the
        # attention units, one 512-col matmul granule per mc step, so a
        # ~0.5us slice of independent PE work sits between every attn@v
        # and the next scores matmul (absorbing the exp psum ping-pong
        # wait that otherwise stalls the PE).
        def proj_granules(p, w_sb, src, dst, sc):
            """One granule = both 512-col j halves of one kc chunk: the two
            matmuls share a stationary, so the second's LDWEIGHTS dedupes to
            ~19ns instead of paying the ~100ns weight-FIFO exposure."""
            state = {}
            def granule(kc, state=state, p=p, w_sb=w_sb, src=src, dst=dst, sc=sc):
                if kc == 0:
                    state["ps"] = popool.tile([128, 1024], F32, tag="po", name="ps_fq")
                ps = state["ps"]
                for j in range(2):
                    nc.tensor.matmul(
                        ps[:, j * 512:(j + 1) * 512],
                        w_sb[:, kc, p * 128:(p + 1) * 128],
                        src[:, kc, sc * 1024 + j * 512: sc * 1024 + (j + 1) * 512],
                        start=(kc == 0),
                        stop=(kc == KC - 1),
                    )
                if kc == KC - 1:
                    nc.vector.tensor_copy(dst[:, sc * 1024:(sc + 1) * 1024], ps[:])
            return [lambda kc=kc: granule(kc) for kc in range(KC)]

        # unit-0 fill stream, in DMA arrival order: vp 4-7, khT sc1, vp 8-15
        pro_fills = []
        for mc in range(4, 8):
            pro_fills += vp_granules(mc)
        pro_fills += proj_granules(0, wk_sb, kT_sb, khT_sb[0], 1)
        for mc in range(8, MC):
            pro_fills += vp_granules(mc)

        # fq_early: projections units 2-4 need soon; fq_mid: qhT pair-1 sc1
        # is not read until unit 6, so it drains in the otherwise
        # fill-starved units 4-5 instead of adding to the PE-bound early
        # units; fq_late: outproj halves appended as dances unlock them
        fq_early = (
            proj_granules(0, wq_sb, qT_sb, qhT_sb[0], 1)
            + proj_granules(1, wk_sb, kT_sb, khT_sb[1], 1)
            + proj_granules(1, wq_sb, qT_sb, qhT_sb[1], 0)
        )
        fq_mid = proj_granules(1, wq_sb, qT_sb, qhT_sb[1], 1)
        fq_late = []

        # ---- attention: flat software-pipelined stream -------------------
        # Per unit (head, q-range): 16 scores+exp iterations; attn@v lags
        # the exp stream by 2 so an exp wait never sits between a scores
        # matmul and the next unit's scores.  The last two attn@v matmuls
        # and the normalization of unit u are carried into unit u+1's first
        # slots (pout is double-buffered, so no serialization).
        def emit_av(st, mc):
            p, hh, h = st["p"], st["hh"], 2 * st["p"] + st["hh"]
            qw = st["qw"]
            for j in range((qw + 511) // 512):
                w = min(512, qw - j * 512)
                nc.tensor.matmul(
                    st["pout"][:, j * 512:j * 512 + w],
                    vh_sb[:, mc, h, :],
                    st["at"][mc][:, j * 512:j * 512 + w],
                    start=(mc == 0),
                    stop=(mc == MC - 1),
                )
            del st["at"][mc]

        def emit_dance(st):
            p, hh = st["p"], st["hh"]
            hlo, hhi = hh * 64, hh * 64 + 64
            qlo, qw = st["qlo"], st["qw"]
            pout = st["pout"]
            sums = dance.tile([1, qw], F32, tag="sums", name="sums")
            nc.vector.tensor_copy(sums[:], pout[64:65, :])
            rcp32 = dance.tile([1, qw], F32, tag="rcp32", name="rcp32")
            nc.vector.reciprocal_approx_fast(rcp32[:], sums[:])
            rcpb = dance.tile([64, qw], F32, tag="rcpb", name="rcpb")
            nc.gpsimd.partition_broadcast(rcpb[:], rcp32[:])
            nc.vector.tensor_tensor(
                outT2_sb[p][hlo:hhi, qlo:qlo + qw],
                pout[0:64, :],
                rcpb[:],
                mybir.AluOpType.mult,
            )

        def attention_unit(p, hh, qlo, qw, fills, carry, unit_idx,
                           pop_schedule=None):
            st = {
                "p": p, "hh": hh, "qlo": qlo, "qw": qw, "at": {},
                "pout": popool.tile([65, qw], F32, tag="po", name="pout"),
            }
            for mc in range(MC):
                # scores first so exp(mc) can start as early as possible;
                # the previous unit's carried attn@v tail follows (it only
                # gates pout/attnT slots needed a couple of steps later)
                ps = ppool.tile([128, qw], F32, tag="pp", name="ps_sc")
                for j in range((qw + 511) // 512):
                    w = min(512, qw - j * 512)
                    nc.tensor.matmul(
                        ps[:, j * 512:j * 512 + w],
                        khT_sb[p][hh * 64:hh * 64 + 64, mc * 128:(mc + 1) * 128],
                        qhT_sb[p][hh * 64:hh * 64 + 64,
                                  qlo + j * 512: qlo + j * 512 + w],
                        start=True,
                        stop=True,
                    )
                at = att.tile([128, qw], BF16, tag="attnT", name="at")
                nc.scalar.activation(at[:], ps[:], AFT.Exp, scale=0.125)
                st["at"][mc] = at
                for _ in range(2):
                    if carry:
                        carry.pop(0)()
                if mc >= 2:
                    emit_av(st, mc - 2)
                # fills only from mc 2 on: a fill popped at mc 0/1 could read
                # outT2 whose producing dance is carried into this unit's
                # mc 1 slot and so is not yet emitted (stale read)
                if pop_schedule is not None:
                    n_pops = pop_schedule(mc)
                elif mc >= 2:
                    # double pop at mc 15: the extra fill absorbs the
                    # unit-boundary wait on the previous unit's exp tail
                    n_pops = 2 if mc == 15 else 1
                else:
                    n_pops = 0
                for _ in range(n_pops):
                    if fills:
                        fills.pop(0)()
            return [
                lambda: emit_av(st, MC - 2),
                lambda: emit_av(st, MC - 1),
                lambda: emit_dance(st),
            ]

        # pair-outer unit order: pair-1 attention starts at unit 4, so its
        # projections drain as fills through units 0-3.  The last 1024-col
        # unit is split into two 512-col halves so the first half's dance
        # unlocks outproj qi 8-11 while the second half still runs.
        units = [(0, 0, 0, 1024), (0, 1, 0, 1024),
                 (0, 0, 1024, 1024), (0, 1, 1024, 1024),
                 (1, 0, 0, 1024), (1, 1, 0, 1024),
                 (1, 0, 1024, 1024),
                 (1, 1, 1024, 512), (1, 1, 1536, 256), (1, 1, 1792, 256)]
        def u0_pops(mc):
            if mc < 2:
                return 0
            return 4 if mc < 4 else 2

        def mid_pops(mc):
            return 1 if mc in (3, 7, 11, 14) else 0

        carry = []
        for u, (p, hh, qlo, qw) in enumerate(units):
            if u == 0:
                fills, sched = pro_fills, u0_pops
            elif u <= 3:
                fills, sched = fq_early, None
            elif u <= 5:
                fills, sched = fq_mid, mid_pops
            else:
                fills, sched = fq_late, None
            carry = attention_unit(p, hh, qlo, qw, fills, carry, u,
                                   pop_schedule=sched)
            if u == 5:
                for qi in range(0, 8):
                    for j in range(2):
                        fq_late.append(
                            lambda qi=qi, j=j: emit_outproj_half(qi, j)
                        )
            if u == 7:
                for qi in range(8, 12):
                    for j in range(2):
                        fq_late.append(
                            lambda qi=qi, j=j: emit_outproj_half(qi, j)
                        )
            if u == 8:
                for qi in range(12, 14):
                    for j in range(2):
                        fq_late.append(
                            lambda qi=qi, j=j: emit_outproj_half(qi, j)
                        )
        for f in carry:
            f()
        for q in (fq_early, fq_mid, fq_late):
            while q:
                q.pop(0)()
        for qi in range(14, 16):
            for j in range(2):
                emit_outproj_half(qi, j, use_act=(j == 1))


def build_program():
    nc = bacc.Bacc(
        "TRN2",
        target_bir_lowering=False,
        debug=False,
        enable_asserts=False,
        num_devices=N_CORES,
    )
    qT = nc.dram_tensor("qT", [D, S], BF16, kind="ExternalInput")
    kT = nc.dram_tensor("kT", [D, S], BF16, kind="ExternalInput")
    vT = nc.dram_tensor("vT", [D, S], BF16, kind="ExternalInput")
    wq = nc.dram_tensor("wq", [D, HPC * DK], BF16, kind="ExternalInput")
    wk = nc.dram_tensor("wk", [D, HPC * DK], BF16, kind="ExternalInput")
    wv = nc.dram_tensor("wv", [D, HPC * DK], BF16, kind="ExternalInput")
    wo = nc.dram_tensor("wo", [HPC * DK, D], BF16, kind="ExternalInput")
    out = nc.dram_tensor("out", [S, D], BF16, kind="ExternalOutput")
    with tile.TileContext(nc) as tc:
        _emit(tc, qT, kT, vT, wq, wk, wv, wo, out)
    nc.compile()
    return nc


def _get_program():
    if "nc" not in _COMPILED:
        _COMPILED["nc"] = build_program()
    return _COMPILED["nc"]


def make_in_maps(q, k, v, Wq, Wk, Wv, Wo):
    """Shard FULL fp32 inputs into per-core bf16 input maps."""
    q, k, v = (np.asarray(x, np.float32) for x in (q, k, v))
    Wq, Wk, Wv, Wo = (np.asarray(x, np.float32) for x in (Wq, Wk, Wv, Wo))
    qT = [np.ascontiguousarray(q[b].T).astype(BF16_NP) for b in range(B)]
    kT = [np.ascontiguousarray(k[b].T).astype(BF16_NP) for b in range(B)]
    vT = [np.ascontiguousarray(v[b].T).astype(BF16_NP) for b in range(B)]
    in_maps = []
    for c in range(N_CORES):
        b, g = divmod(c, N_CORES // B)
        heads = range(HPC * g, HPC * g + HPC)
        wq_c = np.concatenate([Wq[h] for h in heads], axis=1).astype(BF16_NP)
        wk_c = np.concatenate([Wk[h] for h in heads], axis=1).astype(BF16_NP)
        wv_c = np.concatenate([Wv[h] for h in heads], axis=1).astype(BF16_NP)
        wo_c = np.concatenate(
            [Wo[h * DK:(h + 1) * DK] for h in heads], axis=0
        ).astype(BF16_NP)
        in_maps.append({
            "qT": qT[b], "kT": kT[b], "vT": vT[b],
            "wq": np.ascontiguousarray(wq_c),
            "wk": np.ascontiguousarray(wk_c),
            "wv": np.ascontiguousarray(wv_c),
            "wo": np.ascontiguousarray(wo_c),
        })
    return in_maps


def run_on_hw(in_maps, trace=False):
    nc = _get_program()
    return bass_utils.run_bass_kernel_spmd(
        nc, in_maps, list(range(N_CORES)), trace=trace
    )


def kernel(q, k, v, Wq, Wk, Wv, Wo, bo):
    in_maps = make_in_maps(q, k, v, Wq, Wk, Wv, Wo)
    res = run_on_hw(in_maps)
    bo = np.asarray(bo, np.float32)
    parts = [np.asarray(r["out"]).astype(np.float32) for r in res.results]
    out = np.empty((B, S, D), np.float32)
    per_b = N_CORES // B
    for b in range(B):
        out[b] = np.sum(parts[b * per_b:(b + 1) * per_b], axis=0) + bo
    return out

